# revision 8
# baseline (speedup 1.0000x reference)
"""Trainium2 Bass kernel for EnetGnn (gnn_message_passing).

Strategy (data-parallel over batch N=8 across 8 cores, one sample per core):
  1. Median-pool 8x8 blocks of (x, y, depth) channels via DVE max/match_replace
     rounds (exact rank-31 of 64). Medians kept negated (distances are
     sign-invariant).
  2. KNN as a threshold mask, never materializing indices:
       e_neg[i,j] = 2*(p_i . p_j) - |p_j|^2   (monotone in -D^2 per row)
       teneg_i = 16th largest of row i  (max + match_replace + max)
       A[j,i] = (e_neg[j-orient] >= teneg_i)  in fp16 {0,1}, staged in DRAM
     The per-neighbor MLP commutes with the gather (row-wise ops), so
       mean_k gh[knn[i,k]] = (1/16) * A_row_i . gh   -> dense fp16 matmuls.
  3. GNN iterations: g-MLP on 2700 rows (not 43200), PE transposes of gh,
     adjacency matmul for mT (A streamed from DRAM), fused q update.
     Everything feature-major [C, HW].
  4. 3x3 conv as 9 shifted matmuls over zero-padded fp16 tiles.
"""
import numpy as np
import concourse.bass as bass
import concourse.bacc as bacc
import concourse.mybir as mybir
import concourse.tile as tile
from concourse.bass_utils import run_bass_kernel_spmd

F32 = mybir.dt.float32
F16 = mybir.dt.float16
AF = mybir.ActivationFunctionType
ALU = mybir.AluOpType

N, C, H, W = 8, 128, 45, 60
HW = H * W                      # 2700
K = 16
NEG_INF = -3.0e38

# free-dim chunks of 2700 (PSUM bank = 512 fp32)
CHUNKS = [(0, 512), (512, 512), (1024, 512), (1536, 512), (2048, 512), (2560, 140)]
# partition tiles of 2700
PTILES = [(t * 128, 128) for t in range(21)] + [(2688, 12)]
# conv output row-chunks (rows of 60, <=512 psum floats)
RCHUNKS = [(0, 7), (7, 7), (14, 7), (21, 7), (28, 7), (35, 7), (42, 3)]

_cache = {}


def _ensure_ntff_hook():
    """The container's antenv lacks axon_hooks; synthesize it and register the
    ctypes NTFF profile hook from trn_agent_boot so trace=True works."""
    import sys
    import types
    try:
        from antenv.axon_hooks import get_axon_ntff_profile_hook  # noqa: F401
        return
    except ImportError:
        pass
    try:
        mod = types.ModuleType("antenv.axon_hooks")
        mod._hook = None

        def set_axon_ntff_profile_hook(h):
            mod._hook = h

        def get_axon_ntff_profile_hook():
            return mod._hook

        mod.set_axon_ntff_profile_hook = set_axon_ntff_profile_hook
        mod.get_axon_ntff_profile_hook = get_axon_ntff_profile_hook
        sys.modules["antenv.axon_hooks"] = mod
        import antenv
        antenv.axon_hooks = mod
        from trn_agent_boot.trn_boot import _ntff_profile_via_ctypes
        hook = _ntff_profile_via_ctypes("/opt/axon/libaxon_pjrt.so")
        if hook is not None:
            mod.set_axon_ntff_profile_hook(hook)
    except Exception as e:  # profiling is best-effort
        print(f"ntff hook injection failed: {e}")


def _build(a0, a1, qa):
    nc = bacc.Bacc("TRN2", target_bir_lowering=False, debug=False, num_devices=8)

    h0_d = nc.dram_tensor("h0", (C, HW), F32, kind="ExternalInput")
    psrc_d = nc.dram_tensor("psrc", (3, 8 * H, 8 * W), F32, kind="ExternalInput")
    gw0_d = nc.dram_tensor("gw0T", (C, C), F32, kind="ExternalInput")
    gw1_d = nc.dram_tensor("gw1T", (C, C), F32, kind="ExternalInput")
    qw1_d = nc.dram_tensor("qw1T", (C, C), F32, kind="ExternalInput")
    qw2_d = nc.dram_tensor("qw2T", (C, C), F32, kind="ExternalInput")
    cw_d = nc.dram_tensor("convwT", (C, 18, C), F16, kind="ExternalInput")
    bias_d = nc.dram_tensor("biases", (C, 4), F32, kind="ExternalInput")
    ident_d = nc.dram_tensor("ident", (C, C), F32, kind="ExternalInput")
    out_d = nc.dram_tensor("out", (C, HW), F32, kind="ExternalOutput")

    with tile.TileContext(nc) as tc:
        with tc.tile_pool(name="sb", bufs=1) as sb, \
             tc.tile_pool(name="work", bufs=2) as work, \
             tc.tile_pool(name="ps", bufs=3, space="PSUM") as ps, \
             tc.tile_pool(name="ps2", bufs=2, space="PSUM") as ps2, \
             tc.tile_pool(name="dram", bufs=1, space="DRAM") as dram:

            projn_d = dram.tile([3, HW], F32, tag="projn_d")
            msq_d = dram.tile([1, HW], F32, tag="msq_d")
            teneg_d = dram.tile([1, HW], F32, tag="teneg_d")
            A_d = dram.tile([HW, HW], F16, tag="A_d")

            # ---------------- median pooling (negated medians) ----------------
            psrc_r = psrc_d.rearrange("c (by dy) (bx dx) -> c by bx dy dx", dy=8, dx=8)
            for ch in range(3):
                for t in range(23):
                    nrow = 2 if t < 22 else 1
                    nb = 60 * nrow
                    blk = work.tile([120, 64], F32, tag="blk")
                    for r2 in range(nrow):
                        nc.sync.dma_start(blk[60 * r2:60 * (r2 + 1), :], psrc_r[ch, 2 * t + r2])
                    nc.scalar.activation(blk[:nb], blk[:nb], AF.Copy, scale=-1.0)
                    mm8 = work.tile([120, 8], F32, tag="mm8")
                    for rnd in range(3):
                        nc.vector.max(mm8[:nb], blk[:nb])
                        nc.vector.match_replace(blk[:nb], mm8[:nb], blk[:nb], NEG_INF)
                    nc.vector.max(mm8[:nb], blk[:nb])
                    nc.sync.dma_start(projn_d[ch, 120 * t:120 * t + nb], mm8[:nb, 7:8])

            # ---------------- proj / sq prep ----------------
            proj3 = sb.tile([3, HW], F32, tag="proj3")
            nc.sync.dma_start(proj3[:], projn_d[:])
            sq3 = work.tile([3, HW], F32, tag="row27")
            nc.scalar.activation(sq3[:], proj3[:], AF.Square)
            ones3 = sb.tile([3, 1], F32, tag="ones3")
            nc.vector.memset(ones3[:], 1.0)
            sqr = work.tile([1, HW], F32, tag="row27")
            for c0, ncn in CHUNKS:
                sp = ps.tile([C, 512], F32, tag="mm512")
                nc.tensor.matmul(sp[0:1, :ncn], ones3[:], sq3[:, c0:c0 + ncn], start=True, stop=True)
                nc.scalar.activation(sqr[0:1, c0:c0 + ncn], sp[0:1, :ncn], AF.Copy)
            msqr = work.tile([1, HW], F32, tag="row27")
            nc.scalar.activation(msqr[:], sqr[:], AF.Copy, scale=-1.0)
            nc.sync.dma_start(msq_d[:], msqr[:])

            ones1 = sb.tile([1, C], F32, tag="ones1")
            nc.vector.memset(ones1[:], 1.0)
            sq_b = sb.tile([C, HW], F32, tag="bcast", bufs=1)
            for c0, ncn in CHUNKS:
                bp = ps.tile([C, 512], F32, tag="mm512")
                nc.tensor.matmul(bp[:, :ncn], ones1[:], sqr[0:1, c0:c0 + ncn], start=True, stop=True)
                nc.scalar.activation(sq_b[:, c0:c0 + ncn], bp[:, :ncn], AF.Copy)

            # ---------------- phase 1: per-row 16th-largest thresholds ----------------
            for i0, ni in PTILES:
                en = work.tile([C, HW], F32, tag="en")
                for c0, ncn in CHUNKS:
                    rp = ps.tile([C, 512], F32, tag="mm512")
                    nc.tensor.matmul(rp[:ni, :ncn], proj3[:, i0:i0 + ni], proj3[:, c0:c0 + ncn],
                                     start=True, stop=True)
                    nc.vector.scalar_tensor_tensor(en[:ni, c0:c0 + ncn], rp[:ni, :ncn], 2.0,
                                                   sq_b[:ni, c0:c0 + ncn], ALU.mult, ALU.subtract)
                m1 = work.tile([C, 8], F32, tag="m1")
                m2 = work.tile([C, 8], F32, tag="m2")
                nc.vector.max(m1[:ni], en[:ni])
                nc.vector.match_replace(en[:ni], m1[:ni], en[:ni], NEG_INF)
                nc.vector.max(m2[:ni], en[:ni])
                nc.sync.dma_start(teneg_d[0, i0:i0 + ni], m2[:ni, 7:8])

            # ---------------- threshold broadcast ----------------
            trow = work.tile([1, HW], F32, tag="row27")
            nc.sync.dma_start(trow[:], teneg_d[:])
            te_b = sb.tile([C, HW], F32, tag="bcast", bufs=1)
            for c0, ncn in CHUNKS:
                bp = ps.tile([C, 512], F32, tag="mm512")
                nc.tensor.matmul(bp[:, :ncn], ones1[:], trow[0:1, c0:c0 + ncn], start=True, stop=True)
                nc.scalar.activation(te_b[:, c0:c0 + ncn], bp[:, :ncn], AF.Copy)

            # ---------------- phase 2: adjacency mask tiles -> DRAM (fp16 {0,1}) -------
            for jt, (j0, nj) in enumerate(PTILES):
                msqc = work.tile([C, 1], F32, tag="msqc")
                nc.sync.dma_start(msqc[:nj], msq_d[0, j0:j0 + nj])
                ep = work.tile([C, HW], F32, tag="en")
                for c0, ncn in CHUNKS:
                    rp = ps.tile([C, 512], F32, tag="mm512")
                    nc.tensor.matmul(rp[:nj, :ncn], proj3[:, j0:j0 + nj], proj3[:, c0:c0 + ncn],
                                     start=True, stop=True)
                    nc.scalar.activation(ep[:nj, c0:c0 + ncn], rp[:nj, :ncn], AF.Identity,
                                         bias=msqc[:nj], scale=2.0)
                Ao = work.tile([C, HW], F16, tag="Aout")
                nc.vector.tensor_tensor(Ao[:nj], ep[:nj], te_b[:nj], ALU.is_ge)
                nc.sync.dma_start(A_d[j0:j0 + nj, :], Ao[:nj])

            # ---------------- phase 3: GNN iterations ----------------
            h0 = sb.tile([C, HW], F32, tag="h0")
            nc.sync.dma_start(h0[:], h0_d[:])
            gw0 = sb.tile([C, C], F32, tag="gw0")
            nc.sync.dma_start(gw0[:], gw0_d[:])
            gw1 = sb.tile([C, C], F32, tag="gw1")
            nc.sync.dma_start(gw1[:], gw1_d[:])
            qw1 = sb.tile([C, C], F32, tag="qw1")
            nc.sync.dma_start(qw1[:], qw1_d[:])
            qw2 = sb.tile([C, C], F32, tag="qw2")
            nc.sync.dma_start(qw2[:], qw2_d[:])
            cw = sb.tile([C, 18, C], F16, tag="cw")
            nc.sync.dma_start(cw[:], cw_d[:])
            bia = sb.tile([C, 4], F32, tag="bias")
            nc.sync.dma_start(bia[:], bias_d[:])
            ident = sb.tile([C, C], F32, tag="ident")
            nc.sync.dma_start(ident[:], ident_d[:])

            ghrm = [sb.tile([PTILES[jt][1], C], F16, tag=f"gr{jt}", name=f"gr{jt}")
                    for jt in range(22)]

            hin = h0
            for it2 in range(2):
                gh2 = work.tile([C, HW], F32, tag="big", bufs=1)
                for c0, ncn in CHUNKS:
                    g1p = ps.tile([C, 512], F32, tag="mm512")
                    nc.tensor.matmul(g1p[:, :ncn], gw0[:], hin[:, c0:c0 + ncn], start=True, stop=True)
                    gh1c = work.tile([C, 512], F32, tag="c512", bufs=3)
                    nc.scalar.activation(gh1c[:, :ncn], g1p[:, :ncn], AF.Prelu,
                                         bias=bia[:, 0:1], alpha=a0)
                    g2p = ps.tile([C, 512], F32, tag="mm512")
                    nc.tensor.matmul(g2p[:, :ncn], gw1[:], gh1c[:, :ncn], start=True, stop=True)
                    nc.scalar.activation(gh2[:, c0:c0 + ncn], g2p[:, :ncn], AF.Prelu,
                                         bias=bia[:, 1:2], alpha=a1)
                for jt, (j0, nj) in enumerate(PTILES):
                    tp = ps2.tile([C, C], F32, tag="tr")
                    nc.tensor.transpose(tp[:nj], gh2[:, j0:j0 + nj], ident[:])
                    nc.scalar.activation(ghrm[jt][:], tp[:nj], AF.Copy)
                hout = work.tile([C, HW], F32, tag="h")
                for c0, ncn in CHUNKS:
                    mp = ps.tile([C, 512], F32, tag="mm512")
                    for jt, (j0, nj) in enumerate(PTILES):
                        Ain = work.tile([C, 512], F16, tag="Ain", bufs=4,
                                        name=f"Ain_{it2}_{c0}_{jt}")
                        nc.sync.dma_start(Ain[:nj, :ncn], A_d[j0:j0 + nj, c0:c0 + ncn])
                        nc.tensor.matmul(mp[:, :ncn], ghrm[jt][:], Ain[:nj, :ncn],
                                         start=(jt == 0), stop=(jt == 21))
                    mts = work.tile([C, 512], F32, tag="c512", bufs=3)
                    nc.scalar.activation(mts[:, :ncn], mp[:, :ncn], AF.Copy)
                    qp = ps.tile([C, 512], F32, tag="mm512")
                    nc.tensor.matmul(qp[:, :ncn], qw1[:], hin[:, c0:c0 + ncn], start=True, stop=False)
                    nc.tensor.matmul(qp[:, :ncn], qw2[:], mts[:, :ncn], start=False, stop=True)
                    nc.scalar.activation(hout[:, c0:c0 + ncn], qp[:, :ncn], AF.Prelu,
                                         bias=bia[:, 2:3], alpha=qa)
                hin = hout

            # ---------------- conv 3x3 (9 shifted matmuls, fp16) ----------------
            pads = []
            for kh, src in ((0, h0), (1, hin)):
                pad = work.tile([C, H + 2, W + 2], F16, tag="pad", name=f"pad{kh}")
                nc.vector.memset(pad[:], 0.0)
                nc.scalar.activation(pad[:, 1:H + 1, 1:W + 1],
                                     src[:].rearrange("p (h w) -> p h w", h=H), AF.Copy)
                pads.append(pad)
            for r0, nr in RCHUNKS:
                cp = ps2.tile([C, 420], F32, tag="conv")
                first = True
                for dy in range(3):
                    for dx in range(3):
                        for kh in range(2):
                            idx = (dy * 3 + dx) * 2 + kh
                            last = (dy == 2 and dx == 2 and kh == 1)
                            nc.tensor.matmul(cp[:, :nr * W], cw[:, idx, :],
                                             pads[kh][:, r0 + dy:r0 + dy + nr, dx:dx + W],
                                             start=first, stop=last)
                            first = False
                ocs = work.tile([C, 512], F32, tag="c512", bufs=3, name=f"ocs{r0}")
                nc.scalar.activation(ocs[:, :nr * W], cp[:, :nr * W], AF.Identity,
                                     bias=bia[:, 3:4])
                nc.sync.dma_start(out_d[:, r0 * W:(r0 + nr) * W], ocs[:, :nr * W])

    nc.compile()
    return nc


def kernel(cnn_encoder_output, original_input, xy,
           g_w0, g_b0, g_a0, g_w1, g_b1, g_a1,
           q_w, q_b, q_a, conv_w, conv_b,
           gnn_iterations, k, use_half_precision, _trace=False):
    assert int(gnn_iterations) == 2 and int(k) == 16 and int(use_half_precision) == 0

    cnn = np.ascontiguousarray(np.asarray(cnn_encoder_output, dtype=np.float32))
    orig = np.asarray(original_input, dtype=np.float32)
    xy = np.asarray(xy, dtype=np.float32)
    a0, a1, qa = float(np.ravel(g_a0)[0]), float(np.ravel(g_a1)[0]), float(np.ravel(q_a)[0])

    key = (a0, a1, qa)
    if key not in _cache:
        _cache[key] = _build(a0, a1, qa)
    nc = _cache[key]

    g_w0 = np.asarray(g_w0, np.float32)
    g_w1 = np.asarray(g_w1, np.float32)
    q_w = np.asarray(q_w, np.float32)
    conv_w = np.asarray(conv_w, np.float32)

    gw0T = np.ascontiguousarray(g_w0.T)
    gw1T = np.ascontiguousarray(g_w1.T)
    qw1T = np.ascontiguousarray(q_w[:, :C].T)
    qw2T = np.ascontiguousarray(q_w[:, C:].T / float(K))
    # convwT[cin_half, (dy*3+dx)*2+kh, cout] = conv_w[cout, kh*128+cin_half, dy, dx]
    cwT = np.empty((C, 18, C), np.float16)
    for dy in range(3):
        for dx in range(3):
            for kh in range(2):
                idx = (dy * 3 + dx) * 2 + kh
                cwT[:, idx, :] = conv_w[:, kh * C:(kh + 1) * C, dy, dx].T.astype(np.float16)
    biases = np.stack([np.asarray(g_b0, np.float32), np.asarray(g_b1, np.float32),
                       np.asarray(q_b, np.float32), np.asarray(conv_b, np.float32)], axis=1)
    ident = np.eye(C, dtype=np.float32)

    shared = dict(gw0T=gw0T, gw1T=gw1T, qw1T=qw1T, qw2T=qw2T, convwT=cwT,
                  biases=np.ascontiguousarray(biases), ident=ident)
    in_maps = []
    for n in range(N):
        psrc = np.stack([xy[n, 0], xy[n, 1], orig[n, 3]], axis=0)
        in_maps.append(dict(h0=np.ascontiguousarray(cnn[n].reshape(C, HW)),
                            psrc=np.ascontiguousarray(psrc), **shared))

    if _trace:
        _ensure_ntff_hook()
    res = run_bass_kernel_spmd(nc, in_maps, core_ids=list(range(N)), trace=_trace,
                               trace_cores=list(range(N)) if _trace else None)
    out = np.stack([res.results[n]["out"].reshape(C, H, W) for n in range(N)])
    if _trace:
        kernel._last_results = res
    return out


# revision 10
# speedup vs baseline: 1.7061x; 1.7061x over previous
"""Trainium2 Bass kernel for EnetGnn (gnn_message_passing).

Strategy (data-parallel over batch N=8 across 8 cores, one sample per core):
  1. Median-pool 8x8 blocks of (x, y, depth) channels via DVE max/match_replace
     rounds (exact rank-31 of 64). Medians kept negated (distances are
     sign-invariant).
  2. KNN as a threshold mask, never materializing indices:
       e_neg[i,j] = 2*(p_i . p_j) - |p_j|^2   (monotone in -D^2 per row)
       teneg_i = 16th largest of row i  (max + match_replace + max)
       A[j,i] = (e_neg[j-orient] >= teneg_i)  in fp16 {0,1}, staged in DRAM
     The per-neighbor MLP commutes with the gather (row-wise ops), so
       mean_k gh[knn[i,k]] = (1/16) * A_row_i . gh   -> dense fp16 matmuls.
  3. GNN iterations: g-MLP on 2700 rows (not 43200), PE transposes of gh,
     adjacency matmul for mT (A streamed from DRAM), fused q update.
     Everything feature-major [C, HW].
  4. 3x3 conv as 9 shifted matmuls over zero-padded fp16 tiles.

Iter-1 g-MLP / transposes / conv padding prep are emitted first so PE and ACT
work under the DVE-bound median phase (engines execute their streams in order).
"""
import numpy as np
import concourse.bass as bass
import concourse.bacc as bacc
import concourse.mybir as mybir
import concourse.tile as tile
from concourse.bass_utils import run_bass_kernel_spmd

F32 = mybir.dt.float32
F16 = mybir.dt.float16
AF = mybir.ActivationFunctionType
ALU = mybir.AluOpType

N, C, H, W = 8, 128, 45, 60
HW = H * W                      # 2700
K = 16
NEG_INF = -3.0e38

# free-dim chunks of 2700 (PSUM bank = 512 fp32)
CHUNKS = [(0, 512), (512, 512), (1024, 512), (1536, 512), (2048, 512), (2560, 140)]
# chunk pairs for the aggregation matmul (one A DMA covers both)
CPAIRS = [[(0, 512), (512, 512)], [(1024, 512), (1536, 512)], [(2048, 512), (2560, 140)]]
# partition tiles of 2700
PTILES = [(t * 128, 128) for t in range(21)] + [(2688, 12)]
# conv output row-chunks (rows of 60, <=512 psum floats)
RCHUNKS = [(0, 7), (7, 7), (14, 7), (21, 7), (28, 7), (35, 7), (42, 3)]

_cache = {}


def _ensure_ntff_hook():
    """The container's antenv lacks axon_hooks; synthesize it and register the
    ctypes NTFF profile hook from trn_agent_boot so trace=True works."""
    import sys
    import types
    try:
        from antenv.axon_hooks import get_axon_ntff_profile_hook  # noqa: F401
        return
    except ImportError:
        pass
    try:
        mod = types.ModuleType("antenv.axon_hooks")
        mod._hook = None

        def set_axon_ntff_profile_hook(h):
            mod._hook = h

        def get_axon_ntff_profile_hook():
            return mod._hook

        mod.set_axon_ntff_profile_hook = set_axon_ntff_profile_hook
        mod.get_axon_ntff_profile_hook = get_axon_ntff_profile_hook
        sys.modules["antenv.axon_hooks"] = mod
        import antenv
        antenv.axon_hooks = mod
        from trn_agent_boot.trn_boot import _ntff_profile_via_ctypes
        hook = _ntff_profile_via_ctypes("/opt/axon/libaxon_pjrt.so")
        if hook is not None:
            mod.set_axon_ntff_profile_hook(hook)
    except Exception as e:  # profiling is best-effort
        print(f"ntff hook injection failed: {e}")


def _build(a0, a1, qa):
    nc = bacc.Bacc("TRN2", target_bir_lowering=False, debug=False, num_devices=8)

    h0_d = nc.dram_tensor("h0", (C, HW), F32, kind="ExternalInput")
    psrc_d = nc.dram_tensor("psrc", (3, 8 * H, 8 * W), F32, kind="ExternalInput")
    gw0_d = nc.dram_tensor("gw0T", (C, C), F32, kind="ExternalInput")
    gw1_d = nc.dram_tensor("gw1T", (C, C), F32, kind="ExternalInput")
    qw1_d = nc.dram_tensor("qw1T", (C, C), F32, kind="ExternalInput")
    qw2_d = nc.dram_tensor("qw2T", (C, C), F32, kind="ExternalInput")
    cw_d = nc.dram_tensor("convwT", (C, 18, C), F16, kind="ExternalInput")
    bias_d = nc.dram_tensor("biases", (C, 4), F32, kind="ExternalInput")
    ident_d = nc.dram_tensor("ident", (C, C), F32, kind="ExternalInput")
    out_d = nc.dram_tensor("out", (C, HW), F32, kind="ExternalOutput")

    with tile.TileContext(nc) as tc:
        with tc.tile_pool(name="sb", bufs=1) as sb, \
             tc.tile_pool(name="work", bufs=2) as work, \
             tc.tile_pool(name="ps", bufs=3, space="PSUM") as ps, \
             tc.tile_pool(name="ps2", bufs=2, space="PSUM") as ps2, \
             tc.tile_pool(name="dram", bufs=1, space="DRAM") as dram:

            projn_d = dram.tile([3, HW], F32, tag="projn_d")
            msq_d = dram.tile([1, HW], F32, tag="msq_d")
            teneg_d = dram.tile([1, HW], F32, tag="teneg_d")
            A_d = dram.tile([HW, HW], F16, tag="A_d")

            # ---------------- inputs / weights ----------------
            h0 = sb.tile([C, HW], F32, tag="h0")
            nc.sync.dma_start(h0[:], h0_d[:])
            gw0 = sb.tile([C, C], F32, tag="gw0")
            nc.sync.dma_start(gw0[:], gw0_d[:])
            gw1 = sb.tile([C, C], F32, tag="gw1")
            nc.sync.dma_start(gw1[:], gw1_d[:])
            qw1 = sb.tile([C, C], F32, tag="qw1")
            nc.sync.dma_start(qw1[:], qw1_d[:])
            qw2 = sb.tile([C, C], F32, tag="qw2")
            nc.sync.dma_start(qw2[:], qw2_d[:])
            cw = sb.tile([C, 18, C], F16, tag="cw")
            nc.sync.dma_start(cw[:], cw_d[:])
            bia = sb.tile([C, 4], F32, tag="bias")
            nc.sync.dma_start(bia[:], bias_d[:])
            ident = sb.tile([C, C], F32, tag="ident")
            nc.sync.dma_start(ident[:], ident_d[:])

            ghrm = [sb.tile([PTILES[jt][1], C], F16, tag=f"gr{jt}", name=f"gr{jt}")
                    for jt in range(22)]

            def g_mlp(hin, it2):
                """gh2 = prelu(W1 prelu(W0 h + b0) + b1); then ghrm[jt] = gh2^T slices."""
                gh2 = work.tile([C, HW], F32, tag="big", bufs=1, name=f"gh2_{it2}")
                for c0, ncn in CHUNKS:
                    g1p = ps.tile([C, 512], F32, tag="mm512", name=f"g1p_{it2}_{c0}")
                    nc.tensor.matmul(g1p[:, :ncn], gw0[:], hin[:, c0:c0 + ncn], start=True, stop=True)
                    gh1c = work.tile([C, 512], F32, tag="c512", bufs=4, name=f"gh1c_{it2}_{c0}")
                    nc.scalar.activation(gh1c[:, :ncn], g1p[:, :ncn], AF.Prelu,
                                         bias=bia[:, 0:1], alpha=a0)
                    g2p = ps.tile([C, 512], F32, tag="mm512", name=f"g2p_{it2}_{c0}")
                    nc.tensor.matmul(g2p[:, :ncn], gw1[:], gh1c[:, :ncn], start=True, stop=True)
                    nc.scalar.activation(gh2[:, c0:c0 + ncn], g2p[:, :ncn], AF.Prelu,
                                         bias=bia[:, 1:2], alpha=a1)
                for jt, (j0, nj) in enumerate(PTILES):
                    tp = ps2.tile([C, C], F32, tag="tr", name=f"tp_{it2}_{jt}")
                    nc.tensor.transpose(tp[:nj], gh2[:, j0:j0 + nj], ident[:])
                    nc.scalar.activation(ghrm[jt][:], tp[:nj], AF.Copy)

            # iter-1 g-MLP + transposes + conv pad0: depend only on h0/weights,
            # emitted first so PE/ACT run under the DVE-bound median phase.
            g_mlp(h0, 0)
            pads = []
            for kh in range(2):
                pad = work.tile([C, H + 2, W + 2], F16, tag="pad", name=f"pad{kh}")
                nc.vector.memset(pad[:], 0.0)
                pads.append(pad)
            nc.scalar.activation(pads[0][:, 1:H + 1, 1:W + 1],
                                 h0[:].rearrange("p (h w) -> p h w", h=H), AF.Copy)

            # ---------------- median pooling (negated medians) ----------------
            psrc_r = psrc_d.rearrange("c (by dy) (bx dx) -> c by bx dy dx", dy=8, dx=8)
            for ch in range(3):
                for t in range(23):
                    nrow = 2 if t < 22 else 1
                    nb = 60 * nrow
                    blk = work.tile([120, 64], F32, tag="blk", bufs=8)
                    for r2 in range(nrow):
                        nc.sync.dma_start(blk[60 * r2:60 * (r2 + 1), :], psrc_r[ch, 2 * t + r2])
                    nc.scalar.activation(blk[:nb], blk[:nb], AF.Copy, scale=-1.0)
                    mm8 = work.tile([120, 8], F32, tag="mm8", bufs=8)
                    for rnd in range(3):
                        nc.vector.max(mm8[:nb], blk[:nb])
                        nc.vector.match_replace(blk[:nb], mm8[:nb], blk[:nb], NEG_INF)
                    nc.vector.max(mm8[:nb], blk[:nb])
                    nc.sync.dma_start(projn_d[ch, 120 * t:120 * t + nb], mm8[:nb, 7:8])

            # ---------------- proj / sq prep (fp16 proj for PE) ----------------
            proj3 = sb.tile([3, HW], F32, tag="proj3")
            nc.sync.dma_start(proj3[:], projn_d[:])
            p3h = sb.tile([3, HW], F16, tag="p3h")
            nc.scalar.activation(p3h[:], proj3[:], AF.Copy)
            sq3 = work.tile([3, HW], F32, tag="row27")
            nc.scalar.activation(sq3[:], p3h[:], AF.Square)
            ones3 = sb.tile([3, 1], F32, tag="ones3")
            nc.vector.memset(ones3[:], 1.0)
            sqr = work.tile([1, HW], F32, tag="row27")
            for c0, ncn in CHUNKS:
                sp = ps.tile([C, 512], F32, tag="mm512", name=f"sp_{c0}")
                nc.tensor.matmul(sp[0:1, :ncn], ones3[:], sq3[:, c0:c0 + ncn], start=True, stop=True)
                nc.scalar.activation(sqr[0:1, c0:c0 + ncn], sp[0:1, :ncn], AF.Copy)
            msqr = work.tile([1, HW], F32, tag="row27")
            nc.scalar.activation(msqr[:], sqr[:], AF.Copy, scale=-1.0)
            nc.sync.dma_start(msq_d[:], msqr[:])

            ones1 = sb.tile([1, C], F32, tag="ones1")
            nc.vector.memset(ones1[:], 1.0)
            sq_b = sb.tile([C, HW], F32, tag="bcast", bufs=1)
            for c0, ncn in CHUNKS:
                bp = ps.tile([C, 512], F32, tag="mm512", name=f"bp_{c0}")
                nc.tensor.matmul(bp[:, :ncn], ones1[:], sqr[0:1, c0:c0 + ncn], start=True, stop=True)
                nc.scalar.activation(sq_b[:, c0:c0 + ncn], bp[:, :ncn], AF.Copy)

            # ---------------- phase 1: per-row 16th-largest thresholds ----------------
            # PE r -> ACT copy to SBUF -> GPSIMD e_neg=2r-sq -> DVE selection
            for i0, ni in PTILES:
                en = work.tile([C, HW], F32, tag="en")
                for c0, ncn in CHUNKS:
                    rp = ps.tile([C, 512], F32, tag="mm512", name=f"rp1_{i0}_{c0}")
                    nc.tensor.matmul(rp[:ni, :ncn], p3h[:, i0:i0 + ni], p3h[:, c0:c0 + ncn],
                                     start=True, stop=True)
                    rs = work.tile([C, 512], F32, tag="c512r", bufs=4, name=f"rs_{i0}_{c0}")
                    nc.scalar.activation(rs[:ni, :ncn], rp[:ni, :ncn], AF.Copy, scale=2.0)
                    nc.gpsimd.tensor_tensor(en[:ni, c0:c0 + ncn], rs[:ni, :ncn],
                                            sq_b[:ni, c0:c0 + ncn], ALU.subtract)
                m1 = work.tile([C, 8], F32, tag="m1")
                m2 = work.tile([C, 8], F32, tag="m2")
                nc.vector.max(m1[:ni], en[:ni])
                nc.vector.match_replace(en[:ni], m1[:ni], en[:ni], NEG_INF)
                nc.vector.max(m2[:ni], en[:ni])
                nc.sync.dma_start(teneg_d[0, i0:i0 + ni], m2[:ni, 7:8])

            # ---------------- threshold broadcast ----------------
            trow = work.tile([1, HW], F32, tag="row27")
            nc.sync.dma_start(trow[:], teneg_d[:])
            te_b = sb.tile([C, HW], F32, tag="bcast", bufs=1)
            for c0, ncn in CHUNKS:
                bp = ps.tile([C, 512], F32, tag="mm512", name=f"tb_{c0}")
                nc.tensor.matmul(bp[:, :ncn], ones1[:], trow[0:1, c0:c0 + ncn], start=True, stop=True)
                nc.scalar.activation(te_b[:, c0:c0 + ncn], bp[:, :ncn], AF.Copy)

            # ---------------- phase 2: adjacency mask tiles -> DRAM (fp16 {0,1}) -------
            for jt, (j0, nj) in enumerate(PTILES):
                msqc = work.tile([C, 1], F32, tag="msqc")
                nc.sync.dma_start(msqc[:nj], msq_d[0, j0:j0 + nj])
                ep = work.tile([C, HW], F32, tag="en", name=f"ep{jt}")
                for c0, ncn in CHUNKS:
                    rp = ps.tile([C, 512], F32, tag="mm512", name=f"rp2_{jt}_{c0}")
                    nc.tensor.matmul(rp[:nj, :ncn], p3h[:, j0:j0 + nj], p3h[:, c0:c0 + ncn],
                                     start=True, stop=True)
                    nc.scalar.activation(ep[:nj, c0:c0 + ncn], rp[:nj, :ncn], AF.Identity,
                                         bias=msqc[:nj], scale=2.0)
                Ao = work.tile([C, HW], F16, tag="Aout")
                nc.vector.tensor_tensor(Ao[:nj], ep[:nj], te_b[:nj], ALU.is_ge)
                nc.sync.dma_start(A_d[j0:j0 + nj, :], Ao[:nj])

            # ---------------- phase 3: GNN iterations ----------------
            hin = h0
            for it2 in range(2):
                if it2 > 0:
                    g_mlp(hin, it2)
                hout = work.tile([C, HW], F32, tag="h", name=f"h{it2}")
                for pi, pair in enumerate(CPAIRS):
                    pc0 = pair[0][0]
                    pw = sum(ncn for _, ncn in pair)
                    mps = [ps.tile([C, 512], F32, tag="mm512", name=f"mp_{it2}_{pi}_{s}")
                           for s in range(2)]
                    for jt, (j0, nj) in enumerate(PTILES):
                        Ain = work.tile([C, 1024], F16, tag="Ain", bufs=6,
                                        name=f"Ain_{it2}_{pi}_{jt}")
                        nc.sync.dma_start(Ain[:nj, :pw], A_d[j0:j0 + nj, pc0:pc0 + pw])
                        for s, (c0, ncn) in enumerate(pair):
                            nc.tensor.matmul(mps[s][:, :ncn], ghrm[jt][:],
                                             Ain[:nj, c0 - pc0:c0 - pc0 + ncn],
                                             start=(jt == 0), stop=(jt == 21))
                    for s, (c0, ncn) in enumerate(pair):
                        mts = work.tile([C, 512], F32, tag="c512", bufs=4,
                                        name=f"mts_{it2}_{pi}_{s}")
                        nc.scalar.activation(mts[:, :ncn], mps[s][:, :ncn], AF.Copy)
                        qp = ps.tile([C, 512], F32, tag="mm512", name=f"qp_{it2}_{pi}_{s}")
                        nc.tensor.matmul(qp[:, :ncn], qw1[:], hin[:, c0:c0 + ncn],
                                         start=True, stop=False)
                        nc.tensor.matmul(qp[:, :ncn], qw2[:], mts[:, :ncn],
                                         start=False, stop=True)
                        nc.scalar.activation(hout[:, c0:c0 + ncn], qp[:, :ncn], AF.Prelu,
                                             bias=bia[:, 2:3], alpha=qa)
                hin = hout

            # ---------------- conv 3x3 (9 shifted matmuls, fp16) ----------------
            nc.scalar.activation(pads[1][:, 1:H + 1, 1:W + 1],
                                 hin[:].rearrange("p (h w) -> p h w", h=H), AF.Copy)
            for r0, nr in RCHUNKS:
                cp = ps2.tile([C, 420], F32, tag="conv", name=f"cp{r0}")
                first = True
                for dy in range(3):
                    for dx in range(3):
                        for kh in range(2):
                            idx = (dy * 3 + dx) * 2 + kh
                            last = (dy == 2 and dx == 2 and kh == 1)
                            nc.tensor.matmul(cp[:, :nr * W], cw[:, idx, :],
                                             pads[kh][:, r0 + dy:r0 + dy + nr, dx:dx + W],
                                             start=first, stop=last)
                            first = False
                ocs = work.tile([C, 512], F32, tag="c512", bufs=4, name=f"ocs{r0}")
                nc.scalar.activation(ocs[:, :nr * W], cp[:, :nr * W], AF.Identity,
                                     bias=bia[:, 3:4])
                nc.sync.dma_start(out_d[:, r0 * W:(r0 + nr) * W], ocs[:, :nr * W])

    nc.compile()
    return nc


def kernel(cnn_encoder_output, original_input, xy,
           g_w0, g_b0, g_a0, g_w1, g_b1, g_a1,
           q_w, q_b, q_a, conv_w, conv_b,
           gnn_iterations, k, use_half_precision, _trace=False):
    assert int(gnn_iterations) == 2 and int(k) == 16 and int(use_half_precision) == 0

    cnn = np.ascontiguousarray(np.asarray(cnn_encoder_output, dtype=np.float32))
    orig = np.asarray(original_input, dtype=np.float32)
    xy = np.asarray(xy, dtype=np.float32)
    a0, a1, qa = float(np.ravel(g_a0)[0]), float(np.ravel(g_a1)[0]), float(np.ravel(q_a)[0])

    key = (a0, a1, qa)
    if key not in _cache:
        _cache[key] = _build(a0, a1, qa)
    nc = _cache[key]

    g_w0 = np.asarray(g_w0, np.float32)
    g_w1 = np.asarray(g_w1, np.float32)
    q_w = np.asarray(q_w, np.float32)
    conv_w = np.asarray(conv_w, np.float32)

    gw0T = np.ascontiguousarray(g_w0.T)
    gw1T = np.ascontiguousarray(g_w1.T)
    qw1T = np.ascontiguousarray(q_w[:, :C].T)
    qw2T = np.ascontiguousarray(q_w[:, C:].T / float(K))
    # convwT[cin_half, (dy*3+dx)*2+kh, cout] = conv_w[cout, kh*128+cin_half, dy, dx]
    cwT = np.empty((C, 18, C), np.float16)
    for dy in range(3):
        for dx in range(3):
            for kh in range(2):
                idx = (dy * 3 + dx) * 2 + kh
                cwT[:, idx, :] = conv_w[:, kh * C:(kh + 1) * C, dy, dx].T.astype(np.float16)
    biases = np.stack([np.asarray(g_b0, np.float32), np.asarray(g_b1, np.float32),
                       np.asarray(q_b, np.float32), np.asarray(conv_b, np.float32)], axis=1)
    ident = np.eye(C, dtype=np.float32)

    shared = dict(gw0T=gw0T, gw1T=gw1T, qw1T=qw1T, qw2T=qw2T, convwT=cwT,
                  biases=np.ascontiguousarray(biases), ident=ident)
    in_maps = []
    for n in range(N):
        psrc = np.stack([xy[n, 0], xy[n, 1], orig[n, 3]], axis=0)
        in_maps.append(dict(h0=np.ascontiguousarray(cnn[n].reshape(C, HW)),
                            psrc=np.ascontiguousarray(psrc), **shared))

    if _trace:
        _ensure_ntff_hook()
    res = run_bass_kernel_spmd(nc, in_maps, core_ids=list(range(N)), trace=_trace,
                               trace_cores=list(range(N)) if _trace else None)
    out = np.stack([res.results[n]["out"].reshape(C, H, W) for n in range(N)])
    if _trace:
        kernel._last_results = res
    return out
